# revision 3
# baseline (speedup 1.0000x reference)
"""KANLinear (B-spline) Trainium2 kernel — truncated-power formulation, v3.

Math: out = silu(x) @ Wb^T + einsum('nik,oik->no', B(x), Ws*scaler)
t = x/1.2 + 5.5; cardinal B-spline: 6*b_k(t) = sum_r [1,-4,6,-4,1]_r relu(t-k-r)^3.
On-chip: R_j = relu(t-j)^3, S_j = R_j - 2R_{j+1} + R_{j+2} (bf16-safe, <60);
weights absorb the remaining [1,-2,1] filter. |x| < 5.26 (asserted) => t < 9.9:
R_10 = R_11 = 0, S_8 = R_8 - 2R_9, S_9 = R_9.
Sharding: data-parallel over N across 8 cores; weights replicated.
GEMM K-tiles: 4 silu + 10 features * 4 i-tiles = 44, bf16 in, f32 PSUM.
"""
import sys, os
sys.path.insert(0, '/opt/trn_rl_repo')
import numpy as np
import ml_dtypes
from contextlib import ExitStack

import concourse.bass as bass
import concourse.bacc as bacc
import concourse.tile as tile
import concourse.mybir as mybir
from concourse.bass_utils import run_bass_kernel_spmd

f32 = mybir.dt.float32
bf16 = mybir.dt.bfloat16
Alu = mybir.AluOpType
Act = mybir.ActivationFunctionType

N_TOTAL, IN_F, OUT_F = 32768, 512, 512
NCORES = 8
N_CORE = N_TOTAL // NCORES          # 4096
NBLK = 512
NBLOCKS = N_CORE // NBLK            # 8
NJ = 10                             # R_j, j = 0..9
NS = 10                             # S_j features
KT = 4 + NS * 4                     # 44 K-tiles of 128
INV_H = 1.0 / 1.2
T_OFF = 5.5

# engine split knobs (overridable via env KCFG for sweeps)
import json as _json
_KCFG = _json.loads(os.environ.get('KCFG', '{}'))
SQ_ACT_J = tuple(_KCFG.get('sq_act', (0, 1, 2, 3, 4, 5, 6, 7, 8)))  # act Square js
MUL_POOL_J = tuple(_KCFG.get('mul_pool', (8, 9)))           # R_j mul on Pool
S9_ACT = _KCFG.get('s9_act', 0)                             # S9 copy on act
RELU_ORDER = tuple(_KCFG.get('relu_order', tuple(range(10))))
N_DRAIN_ACT = _KCFG.get('drain_act', 2)                     # po drains on act

_cache = {}


def _build():
    if 'nc' in _cache:
        return _cache['nc']
    nc = bacc.Bacc("TRN2", target_bir_lowering=False, debug=False, num_devices=NCORES)
    biases = sorted({T_OFF - j for j in range(NJ)} | {0.0})
    for cv in biases:
        th = nc.alloc_sbuf_tensor(f"constk-{cv}", [128, 1], f32)
        nc.gpsimd.memset(th.ap(), cv)
        nc.const_aps.aps[(f32, cv)] = th.ap()
    nc.all_engine_barrier()
    x_d = nc.dram_tensor("x", [N_CORE, IN_F], bf16, kind="ExternalInput").ap()
    w_d = nc.dram_tensor("w", [KT * 128, OUT_F], bf16, kind="ExternalInput").ap()
    id_d = nc.dram_tensor("ident", [128, 128], bf16, kind="ExternalInput").ap()
    y_d = nc.dram_tensor("y", [N_CORE, OUT_F], bf16, kind="ExternalOutput").ap()

    with tile.TileContext(nc) as tc, ExitStack() as ctx:
        wpool = ctx.enter_context(tc.tile_pool(name="w", bufs=1))
        xpool = ctx.enter_context(tc.tile_pool(name="x", bufs=2))
        rpool = ctx.enter_context(tc.tile_pool(name="r", bufs=2))
        qpool = ctx.enter_context(tc.tile_pool(name="q", bufs=6))
        Rpool = ctx.enter_context(tc.tile_pool(name="R", bufs=2))
        spool = ctx.enter_context(tc.tile_pool(name="S", bufs=2))
        fpool = ctx.enter_context(tc.tile_pool(name="feat", bufs=2))
        ypool = ctx.enter_context(tc.tile_pool(name="y", bufs=2))
        pt_pool = ctx.enter_context(tc.tile_pool(name="ptrans", bufs=4, space="PSUM"))
        po_pool = ctx.enter_context(tc.tile_pool(name="pout", bufs=1, space="PSUM"))

        w_s = wpool.tile([128, KT * OUT_F], bf16, tag="w")
        ident = wpool.tile([128, 128], bf16, tag="ident")
        nc.sync.dma_start(ident[:], id_d[:])
        for kt in range(KT):
            nc.sync.dma_start(w_s[:, kt * OUT_F:(kt + 1) * OUT_F],
                              w_d[kt * 128:(kt + 1) * 128, :])

        NIT = NBLOCKS * 4

        def load_block(blk):
            r0 = blk * NBLK
            xts = []
            for nt in range(4):
                xt = xpool.tile([128, IN_F], bf16, tag=f"xin{nt}", name=f"xin{nt}")
                nc.sync.dma_start(xt[:], x_d[r0 + nt * 128: r0 + (nt + 1) * 128, :])
                xts.append(xt)
            return xts

        def emit_transpose(g, xts_by_blk):
            xts = xts_by_blk[g // 4]
            it = g % 4
            ptr = pt_pool.tile([128, NBLK], bf16, tag="ptr", name="ptr")
            for nt in range(4):
                nc.tensor.transpose(ptr[:, nt * 128:(nt + 1) * 128],
                                    xts[nt][:, it * 128:(it + 1) * 128], ident[:])
            return ptr

        xts_by_blk = {0: load_block(0)}
        ptrs = {0: emit_transpose(0, xts_by_blk), 1: emit_transpose(1, xts_by_blk)}
        po = None

        for g in range(NIT):
            blk, it = g // 4, g % 4
            r0 = blk * NBLK
            if it == 0:
                if blk + 1 < NBLOCKS:
                    xts_by_blk[blk + 1] = load_block(blk + 1)
                    xts_by_blk.pop(blk - 1, None)
                po = [po_pool.tile([128, OUT_F], f32, tag=f"po{nsub}", name=f"po{nsub}")
                      for nsub in range(4)]
            if g + 2 < NIT:
                ptrs[g + 2] = emit_transpose(g + 2, xts_by_blk)
            ptr = ptrs.pop(g)

            # feature buffer (bf16): slots 0..5 = S_j, 6..9 = R_j, 10 = silu
            feat = fpool.tile([128, 11 * NBLK], bf16, tag="feat")
            Rbuf = Rpool.tile([128, 8 * NBLK], f32, tag="Rbuf")
            Stmp = spool.tile([128, 6 * NBLK], f32, tag="Stmp")

            # r_j = relu(x/1.2 + 5.5 - j); emitted per-j right before its q so
            # the act queue yields (r_j, q_j) pairs and R-muls start early
            rts = [None] * NJ

            def emit_r(j):
                rt = rpool.tile([128, NBLK], f32, tag=f"r{j}", name=f"r{j}")
                nc.scalar.activation(rt[:], ptr[:], Act.Relu,
                                     bias=T_OFF - j, scale=INV_H)
                rts[j] = rt

            def emit_R(j):
                # q_j = (t-j)^2; R_j = q_j * r_j.  R_8/R_9 are features
                # themselves: multiply straight into feat (bf16 out).
                qt = qpool.tile([128, NBLK], f32, tag="q", name="q")
                if j in SQ_ACT_J:
                    nc.scalar.activation(qt[:], ptr[:], Act.Square,
                                         bias=T_OFF - j, scale=INV_H)
                else:
                    nc.vector.tensor_mul(qt[:], rts[j][:], rts[j][:])
                if j >= 8:
                    Rslot = feat[:, j * NBLK:(j + 1) * NBLK]
                else:
                    Rslot = Rbuf[:, j * NBLK:(j + 1) * NBLK]
                # NOTE: scalar_tensor_tensor is NOT legal on Pool (neuronxcc
                # rejects TensorScalarPtr with a second tensor input there);
                # only tensor_tensor / tensor_scalar-immediate forms are.
                if j in MUL_POOL_J:
                    nc.gpsimd.tensor_mul(Rslot, qt[:], rts[j][:])
                else:
                    nc.vector.tensor_mul(Rslot, qt[:], rts[j][:])

            def emit_stt(j0, nsl):
                # Stmp_{j0..j0+nsl-1} = R_j - 2 R_{j+1}  (DVE stt, slot-paired)
                nc.vector.scalar_tensor_tensor(
                    Stmp[:, j0 * NBLK:(j0 + nsl) * NBLK],
                    Rbuf[:, (j0 + 1) * NBLK:(j0 + 1 + nsl) * NBLK], -2.0,
                    Rbuf[:, j0 * NBLK:(j0 + nsl) * NBLK], Alu.mult, Alu.add)

            def emit_add(j0, nsl, pool):
                # feat_{j0..} = Stmp_{j0..} + R_{j0+2..}
                Rnext = Rbuf[:, (j0 + 2) * NBLK:(j0 + 2 + nsl) * NBLK]
                Sj = Stmp[:, j0 * NBLK:(j0 + nsl) * NBLK]
                out = feat[:, j0 * NBLK:(j0 + nsl) * NBLK]
                if pool:
                    nc.gpsimd.tensor_add(out, Sj, Rnext)
                else:
                    nc.vector.tensor_add(out, Sj, Rnext)

            for j in RELU_ORDER:
                emit_r(j)
            # emit R/S interleaved so S work starts as soon as inputs exist
            for j in range(NJ):
                emit_R(j)
                if j == 2:
                    emit_stt(0, 2)
                elif j == 3:
                    emit_add(0, 2, False)
                elif j == 4:
                    emit_stt(2, 2)
                elif j == 5:
                    emit_add(2, 1, True)
                    emit_add(3, 1, True)
                elif j == 6:
                    emit_stt(4, 2)
                    # R_6 doubles as feature 6: bf16 convert (Pool ts)
                    nc.gpsimd.tensor_scalar(feat[:, 6 * NBLK:7 * NBLK],
                                            Rbuf[:, 6 * NBLK:7 * NBLK], 1.0,
                                            None, Alu.mult)
                elif j == 7:
                    emit_add(4, 1, True)
                    emit_add(5, 1, False)
                    # R_7 doubles as feature 7
                    nc.gpsimd.tensor_scalar(feat[:, 7 * NBLK:8 * NBLK],
                                            Rbuf[:, 7 * NBLK:8 * NBLK], 1.0,
                                            None, Alu.mult)
            # silu feature
            nc.scalar.activation(feat[:, 10 * NBLK:11 * NBLK], ptr[:], Act.Silu,
                                 bias=0.0, scale=1.0)

            # GEMM: accumulate this it's 11 K-slots into po[0..3]
            for fi in range(11):
                wslot = it if fi == 10 else 4 + fi * 4 + it
                for nsub in range(4):
                    nc.tensor.matmul(
                        po[nsub][:],
                        feat[:, fi * NBLK + nsub * 128: fi * NBLK + (nsub + 1) * 128],
                        w_s[:, wslot * OUT_F:(wslot + 1) * OUT_F],
                        start=(it == 0 and fi == 0), stop=(it == 3 and fi == 10))

            if it == 3:
                for nsub in range(4):
                    yo = ypool.tile([128, OUT_F], bf16, tag=f"yout{nsub}",
                                    name=f"yo{nsub}")
                    if nsub < N_DRAIN_ACT:
                        nc.scalar.copy(yo[:], po[nsub][:])
                    else:
                        nc.vector.tensor_scalar(yo[:], po[nsub][:], 1.0, None,
                                                Alu.mult)
                    nc.sync.dma_start(y_d[r0 + nsub * 128: r0 + (nsub + 1) * 128, :],
                                      yo[:])

    nc.compile()
    _cache['nc'] = nc
    return nc


def _prep_w(base_weight, spline_weight, spline_scaler):
    # b'_k = sum_r [1,-4,6,-4,1]_r R_{k+r}: V_R[o,i,j] is the weight on R_j.
    # On-chip features F = M R with F_j = S_j (j<6) else R_j, so V = M^-T V_R.
    sw6 = (spline_weight * spline_scaler[..., None] / 6.0).astype(np.float64)
    w5 = (1.0, -4.0, 6.0, -4.0, 1.0)
    V_R = np.zeros((OUT_F, IN_F, NS), dtype=np.float64)
    for j in range(NS):
        for k in range(max(0, j - 4), min(7, j) + 1):
            V_R[:, :, j] += w5[j - k] * sw6[:, :, k]
    M = np.zeros((NS, NS), dtype=np.float64)
    for j in range(6):
        M[j, j], M[j, j + 1] = 1.0, -2.0
        if j + 2 < NS:
            M[j, j + 2] = 1.0
    for j in range(6, NS):
        M[j, j] = 1.0
    V = np.einsum('rj,oir->oij', np.linalg.inv(M), V_R).astype(np.float32)
    w = np.zeros((KT * 128, OUT_F), dtype=np.float32)
    w[0:512, :] = base_weight.T                            # silu branch
    for j in range(NS):
        for it in range(4):
            kslot = 4 + j * 4 + it
            w[kslot * 128:(kslot + 1) * 128, :] = V[:, it * 128:(it + 1) * 128, j].T
    return w.astype(ml_dtypes.bfloat16)


def kernel(x, base_weight, spline_weight, spline_scaler, grid):
    x = np.asarray(x, dtype=np.float32)
    # t = x/1.2 + 5.5 must stay below 10 so R_10 = R_11 = 0 (negative x only
    # pushes t below all knots, where every relu clamps to 0 -> b = 0, exact).
    assert x.max() < 5.39, "kernel specialized for x < 5.39 (t < 10)"
    x = x.astype(ml_dtypes.bfloat16)
    w = _prep_w(np.asarray(base_weight, np.float32),
                np.asarray(spline_weight, np.float32),
                np.asarray(spline_scaler, np.float32))
    ident = np.eye(128, dtype=np.float32).astype(ml_dtypes.bfloat16)
    nc = _build()
    in_maps = []
    for c in range(NCORES):
        in_maps.append({"x": np.ascontiguousarray(x[c * N_CORE:(c + 1) * N_CORE]),
                        "w": w, "ident": ident})
    res = run_bass_kernel_spmd(nc, in_maps, core_ids=list(range(NCORES)))
    out = np.concatenate([res.results[c]["y"] for c in range(NCORES)], axis=0)
    return out.astype(np.float32)
